# revision 14
# baseline (speedup 1.0000x reference)
"""MeshLoss2D Trainium2 kernel.

Computes mean over batch of (masked mean over point-cloud points of the
squared distance to the nearest mesh vertex).

Sharding: 8 cores = 4 batches x 2 point-cloud halves. Each core computes
min-squared-distance for its 4096 points against all 8192 vertices of its
batch item. Host applies the zero-column validity mask and the means.

Device math: d2[m,j] = |p_m|^2 - 2 p_m.v_j + |v_j|^2 is computed directly on
the tensor engine as a K=13 augmented matmul. fp32 operands are split into
fp16 hi+lo pairs (hi*hi + hi*lo + lo*hi), which keeps ~fp32 precision while
running the PE at full (1 cycle/row) rate; fp32 matmuls would be 4x slower.
PSUM (fp32) is drained with a min-reduction split across the vector engine
(direct fp32 reduce of one 4-bank quad) and the scalar engine (fp32->fp16
cast-copies of three quads, consumed by fp16 tensor-min ops on the vector
engine at 2x rate).
"""
import sys
import os

sys.path.insert(0, "/opt/trn_rl_repo")

import numpy as np
from contextlib import ExitStack

import concourse.bacc as bacc
import concourse.tile as tile
from concourse import mybir
from concourse.bass_utils import run_bass_kernel_spmd

B = 4
M = 8192          # point-cloud points per batch item
N = 8192          # mesh vertices per batch item (128*64)
NCORES = 8
MQ = M // 2       # points per core
K = 13            # augmented contraction dim
PT = 128          # points per tile (partition dim)
TILES = MQ // PT  # 32
QUAD = 2048       # vertices per PSUM quad (4 banks of 512 fp32)
NQUADS = N // QUAD  # 4

f32 = mybir.dt.float32
f16 = mybir.dt.float16

_NC_CACHE = {}

# Drain configuration: of the 4 PSUM quads per point-tile, how many the
# vector engine reduces directly (fp32) vs. the scalar engine cast-copies to
# fp16 (consumed by fp16 min ops); whether GPSIMD takes the first fp16
# pairwise-min off the vector engine.
CFG = {"direct": 1, "gps": False}


GROUP = 4  # tiles per batched final fp16 reduce


def _build(cfg=None, reps=1, num_devices=NCORES):
    cfg = dict(CFG if cfg is None else cfg)
    key = ("nc", tuple(sorted(cfg.items())), reps, num_devices)
    if key in _NC_CACHE:
        return _NC_CACHE[key]

    nc = bacc.Bacc("TRN2", target_bir_lowering=False, debug=False,
                   enable_asserts=True, num_devices=num_devices)
    lhsT = nc.dram_tensor("lhsT", [K, MQ], f16, kind="ExternalInput")
    rhs = nc.dram_tensor("rhs", [K, N], f16, kind="ExternalInput")
    out = nc.dram_tensor("out", [PT, TILES], f32, kind="ExternalOutput")

    with ExitStack() as ctx:
        tc = ctx.enter_context(tile.TileContext(nc))
        const = ctx.enter_context(tc.tile_pool(name="const", bufs=1))
        ppool = ctx.enter_context(tc.tile_pool(name="ps", bufs=2, space="PSUM"))
        cpool = ctx.enter_context(tc.tile_pool(name="c16", bufs=4))
        c4pool = ctx.enter_context(tc.tile_pool(name="c16w", bufs=2))
        tpool = ctx.enter_context(tc.tile_pool(name="tmp", bufs=4))
        mpool = ctx.enter_context(tc.tile_pool(name="mins", bufs=1))

        lt = const.tile([K, MQ], f16)
        rt = const.tile([K, N], f16)
        nc.sync.dma_start(out=lt, in_=lhsT[:, :])
        nc.sync.dma_start(out=rt, in_=rhs[:, :])

        mins32 = mpool.tile([PT, TILES], f32)
        mins16 = mpool.tile([PT, TILES], f16)
        # all-ACT tiles (see below) never write their mins32 column
        nc.vector.memset(mins32, 1e30)

        def tile_body(t):
            # Load balance: on 3 of 4 tiles the vector engine min-reduces one
            # PSUM quad directly (fp32) while the scalar engine cast-copies
            # the other three to fp16; every 4th tile routes all four quads
            # through the scalar engine, which rebalances the two engines
            # (measured ~5% faster than uniform 1+3).
            allact = (t % 4 == 3)
            ltt = lt[:, t * PT:(t + 1) * PT]
            if not allact:
                # quad 0: fp32 PSUM reduced directly on the vector engine
                q = ppool.tile([PT, QUAD], f32, tag="q")
                for j in range(QUAD // 512):
                    nc.tensor.matmul(q[:, j * 512:(j + 1) * 512], ltt,
                                     rt[:, j * 512:(j + 1) * 512],
                                     start=True, stop=True)
                nc.vector.tensor_reduce(mins32[:, t:t + 1], q,
                                        axis=mybir.AxisListType.X,
                                        op=mybir.AluOpType.min)
            # remaining quads: scalar engine cast-copies PSUM to fp16 SBUF
            nq = NQUADS if allact else NQUADS - 1
            if allact:
                c16 = c4pool.tile([PT, NQUADS, QUAD], f16, tag="c16w")
            else:
                c16 = cpool.tile([PT, NQUADS - 1, QUAD], f16, tag="c16")
            for ci, qi in enumerate(range(0 if allact else 1, NQUADS)):
                q = ppool.tile([PT, QUAD], f32, tag="q")
                for j in range(QUAD // 512):
                    col = qi * QUAD + j * 512
                    nc.tensor.matmul(q[:, j * 512:(j + 1) * 512], ltt,
                                     rt[:, col:col + 512],
                                     start=True, stop=True)
                nc.scalar.copy(out=c16[:, ci, :], in_=q)
            # fp16 min chain on the vector engine (tensor_tensor runs 2x for
            # fp16), then one 1x-rate reduce
            cur = c16[:, 0, :]
            for i in range(1, nq):
                nxt = tpool.tile([PT, QUAD], f16, tag=f"t{i}")
                nc.vector.tensor_tensor(out=nxt, in0=cur, in1=c16[:, i, :],
                                        op=mybir.AluOpType.min)
                cur = nxt
            nc.vector.tensor_reduce(mins16[:, t:t + 1], cur,
                                    axis=mybir.AxisListType.X,
                                    op=mybir.AluOpType.min)

        def whole_pass():
            for t in range(TILES):
                tile_body(t)

        if reps == 1:
            whole_pass()
        else:
            with tc.For_i(0, reps, 1):
                whole_pass()

        m16f = mpool.tile([PT, TILES], f32)
        nc.scalar.copy(out=m16f, in_=mins16)
        both = mpool.tile([PT, TILES], f32)
        nc.vector.tensor_tensor(out=both, in0=mins32, in1=m16f,
                                op=mybir.AluOpType.min)
        nc.sync.dma_start(out=out[:, :], in_=both)

    nc.compile()
    _NC_CACHE[key] = nc
    return nc


def _split16(x):
    hi = x.astype(np.float16)
    lo = (x - hi.astype(np.float32)).astype(np.float16)
    return hi, lo


def _make_in_maps(vertices, pc):
    """vertices [B,3,128,64] f32, pc [B,3,M] f32 -> list of 8 in_maps."""
    in_maps = []
    onesq = np.ones((1, MQ), np.float16)
    onesn = np.ones((1, N), np.float16)
    for b in range(B):
        v = vertices[b].reshape(3, N).astype(np.float32)
        m2v = -2.0 * v
        m2v_hi, m2v_lo = _split16(m2v)
        V2 = (v.astype(np.float64) ** 2).sum(0).astype(np.float32)
        V2_hi, V2_lo = _split16(V2)
        rhs_b = np.concatenate(
            [m2v_hi, m2v_lo, m2v_hi, V2_hi[None], V2_lo[None], onesn, onesn],
            axis=0).astype(np.float16)
        rhs_b = np.ascontiguousarray(rhs_b)
        for h in range(2):
            p = pc[b, :, h * MQ:(h + 1) * MQ].astype(np.float32)
            p_hi, p_lo = _split16(p)
            P2 = (p.astype(np.float64) ** 2).sum(0).astype(np.float32)
            P2_hi, P2_lo = _split16(P2)
            lhsT_c = np.concatenate(
                [p_hi, p_hi, p_lo, onesq, onesq, P2_hi[None], P2_lo[None]],
                axis=0).astype(np.float16)
            in_maps.append({"lhsT": np.ascontiguousarray(lhsT_c),
                            "rhs": rhs_b})
    return in_maps


def _get_runner():
    """Build the kernel once and return a cached callable that executes it
    on all 8 cores via a persistently-jitted shard_map (adapted from
    concourse.bass2jax.run_bass_via_pjrt, which re-jits on every call)."""
    if "runner" in _NC_CACHE:
        return _NC_CACHE["runner"]

    import jax
    from jax.experimental.shard_map import shard_map
    from jax.sharding import Mesh, PartitionSpec
    import concourse.mybir as _mybir
    from concourse import bass2jax

    nc = _build()
    bass2jax.install_neuronx_cc_hook()

    partition_name = nc.partition_id_tensor.name if nc.partition_id_tensor else None
    in_names, out_names, out_avals, zero_shapes = [], [], [], []
    for alloc in nc.m.functions[0].allocations:
        if not isinstance(alloc, _mybir.MemoryLocationSet):
            continue
        name = alloc.memorylocations[0].name
        if alloc.kind == "ExternalInput":
            if name != partition_name:
                in_names.append(name)
        elif alloc.kind == "ExternalOutput":
            shape = tuple(alloc.tensor_shape)
            dtype = _mybir.dt.np(alloc.dtype)
            out_names.append(name)
            out_avals.append(jax.core.ShapedArray(shape, dtype))
            zero_shapes.append((shape, dtype))
    n_params = len(in_names)
    n_outs = len(out_names)
    all_in_names = tuple(in_names + out_names + ([partition_name] if partition_name else []))

    def _body(*args):
        operands = list(args)
        if partition_name is not None:
            operands.append(bass2jax.partition_id_tensor())
        outs = bass2jax._bass_exec_p.bind(
            *operands,
            out_avals=tuple(out_avals),
            in_names=all_in_names,
            out_names=tuple(out_names),
            lowering_input_output_aliases=(),
            sim_require_finite=True,
            sim_require_nnan=True,
            nc=nc,
        )
        return tuple(outs)

    devices = jax.devices()[:NCORES]
    mesh = Mesh(np.asarray(devices), ("core",))
    donate = tuple(range(n_params, n_params + n_outs))
    sharded = jax.jit(
        shard_map(_body, mesh=mesh,
                  in_specs=(PartitionSpec("core"),) * (n_params + n_outs),
                  out_specs=(PartitionSpec("core"),) * n_outs,
                  check_rep=False),
        donate_argnums=donate, keep_unused=True)

    def run(in_maps):
        concat_in = [
            np.concatenate([np.asarray(m[name]) for m in in_maps], axis=0)
            for name in in_names
        ]
        concat_zeros = [
            np.zeros((NCORES * s[0], *s[1:]), d) for (s, d) in zero_shapes
        ]
        out_arrs = jax.block_until_ready(sharded(*concat_in, *concat_zeros))
        return [
            {name: np.asarray(out_arrs[i]).reshape(NCORES, *out_avals[i].shape)[c]
             for i, name in enumerate(out_names)}
            for c in range(NCORES)
        ]

    _NC_CACHE["runner"] = run
    return run


def _run_device(in_maps):
    return _get_runner()(in_maps)


def kernel(vertices, pc):
    vertices = np.asarray(vertices, dtype=np.float32)
    pc = np.asarray(pc, dtype=np.float32)
    in_maps = _make_in_maps(vertices, pc)
    results = _run_device(in_maps)

    dist2 = np.empty((B, M), np.float64)
    for b in range(B):
        for h in range(2):
            core = b * 2 + h
            o = results[core]["out"]              # [128, TILES]
            mins = o.T.reshape(MQ)                # point index = t*128 + m
            dist2[b, h * MQ:(h + 1) * MQ] = mins

    valid = ~np.all(pc == 0.0, axis=1)            # [B, M]
    valid_f = valid.astype(np.float64)
    per_item = (dist2 * valid_f).sum(axis=1) / valid_f.sum(axis=1)
    return np.float32(per_item.mean())


# revision 15
# speedup vs baseline: 1.0782x; 1.0782x over previous
"""MeshLoss2D Trainium2 kernel.

Computes mean over batch of (masked mean over point-cloud points of the
squared distance to the nearest mesh vertex).

Sharding: 8 cores = 4 batches x 2 point-cloud halves. Each core computes
min-squared-distance for its 4096 points against all 8192 vertices of its
batch item. Host applies the zero-column validity mask and the means.

Device math: d2[m,j] = |p_m|^2 - 2 p_m.v_j + |v_j|^2 is computed directly on
the tensor engine as a K=13 augmented matmul. fp32 operands are split into
fp16 hi+lo pairs (hi*hi + hi*lo + lo*hi), which keeps ~fp32 precision while
running the PE at full (1 cycle/row) rate; fp32 matmuls would be 4x slower.
PSUM (fp32) is drained with a min-reduction split across the vector engine
(direct fp32 reduce of one 4-bank quad) and the scalar engine (fp32->fp16
cast-copies of three quads, consumed by fp16 tensor-min ops on the vector
engine at 2x rate).
"""
import sys
import os

sys.path.insert(0, "/opt/trn_rl_repo")

import numpy as np
from contextlib import ExitStack

import concourse.bacc as bacc
import concourse.tile as tile
from concourse import mybir
from concourse.bass_utils import run_bass_kernel_spmd

B = 4
M = 8192          # point-cloud points per batch item
N = 8192          # mesh vertices per batch item (128*64)
NCORES = 8
MQ = M // 2       # points per core
K = 13            # augmented contraction dim
PT = 128          # points per tile (partition dim)
TILES = MQ // PT  # 32
QUAD = 2048       # vertices per PSUM quad (4 banks of 512 fp32)
NQUADS = N // QUAD  # 4

f32 = mybir.dt.float32
f16 = mybir.dt.float16

_NC_CACHE = {}

# Drain configuration: of the 4 PSUM quads per point-tile, how many the
# vector engine reduces directly (fp32) vs. the scalar engine cast-copies to
# fp16 (consumed by fp16 min ops); whether GPSIMD takes the first fp16
# pairwise-min off the vector engine.
CFG = {"direct": 1, "gps": False}


GROUP = 4  # tiles per batched final fp16 reduce


def _build(cfg=None, reps=1, num_devices=NCORES):
    cfg = dict(CFG if cfg is None else cfg)
    key = ("nc", tuple(sorted(cfg.items())), reps, num_devices)
    if key in _NC_CACHE:
        return _NC_CACHE[key]

    nc = bacc.Bacc("TRN2", target_bir_lowering=False, debug=False,
                   enable_asserts=True, num_devices=num_devices)
    lhsT = nc.dram_tensor("lhsT", [K, MQ], f16, kind="ExternalInput")
    rhs = nc.dram_tensor("rhs", [K, N], f16, kind="ExternalInput")
    out = nc.dram_tensor("out", [PT, TILES], f32, kind="ExternalOutput")

    with ExitStack() as ctx:
        tc = ctx.enter_context(tile.TileContext(nc))
        const = ctx.enter_context(tc.tile_pool(name="const", bufs=1))
        ppool = ctx.enter_context(tc.tile_pool(name="ps", bufs=2, space="PSUM"))
        cpool = ctx.enter_context(tc.tile_pool(name="c16", bufs=4))
        c4pool = ctx.enter_context(tc.tile_pool(name="c16w", bufs=2))
        tpool = ctx.enter_context(tc.tile_pool(name="tmp", bufs=4))
        mpool = ctx.enter_context(tc.tile_pool(name="mins", bufs=1))

        lt = const.tile([K, MQ], f16)
        rt = const.tile([K, N], f16)
        # chunked loads so the first tiles' matmuls start before the whole
        # (13-partition, port-inefficient) input DMA completes
        for c in range(0, N, QUAD):
            nc.sync.dma_start(out=rt[:, c:c + QUAD], in_=rhs[:, c:c + QUAD])
        for c in range(0, MQ, 8 * PT):
            nc.sync.dma_start(out=lt[:, c:c + 8 * PT], in_=lhsT[:, c:c + 8 * PT])

        mins32 = mpool.tile([PT, TILES], f32)
        mins16 = mpool.tile([PT, TILES], f16)
        # all-ACT tiles (see below) never write their mins32 column
        nc.vector.memset(mins32, 1e30)

        def tile_body(t):
            # Load balance: on 3 of 4 tiles the vector engine min-reduces one
            # PSUM quad directly (fp32) while the scalar engine cast-copies
            # the other three to fp16; every 4th tile routes all four quads
            # through the scalar engine, which rebalances the two engines
            # (measured ~5% faster than uniform 1+3).
            allact = (t % 4 == 3)
            ltt = lt[:, t * PT:(t + 1) * PT]
            if not allact:
                # quad 0: fp32 PSUM reduced directly on the vector engine
                q = ppool.tile([PT, QUAD], f32, tag="q")
                for j in range(QUAD // 512):
                    nc.tensor.matmul(q[:, j * 512:(j + 1) * 512], ltt,
                                     rt[:, j * 512:(j + 1) * 512],
                                     start=True, stop=True)
                nc.vector.tensor_reduce(mins32[:, t:t + 1], q,
                                        axis=mybir.AxisListType.X,
                                        op=mybir.AluOpType.min)
            # remaining quads: scalar engine cast-copies PSUM to fp16 SBUF
            nq = NQUADS if allact else NQUADS - 1
            if allact:
                c16 = c4pool.tile([PT, NQUADS, QUAD], f16, tag="c16w")
            else:
                c16 = cpool.tile([PT, NQUADS - 1, QUAD], f16, tag="c16")
            for ci, qi in enumerate(range(0 if allact else 1, NQUADS)):
                q = ppool.tile([PT, QUAD], f32, tag="q")
                for j in range(QUAD // 512):
                    col = qi * QUAD + j * 512
                    nc.tensor.matmul(q[:, j * 512:(j + 1) * 512], ltt,
                                     rt[:, col:col + 512],
                                     start=True, stop=True)
                nc.scalar.copy(out=c16[:, ci, :], in_=q)
            # fp16 min chain on the vector engine (tensor_tensor runs 2x for
            # fp16), then one 1x-rate reduce
            cur = c16[:, 0, :]
            for i in range(1, nq):
                nxt = tpool.tile([PT, QUAD], f16, tag=f"t{i}")
                nc.vector.tensor_tensor(out=nxt, in0=cur, in1=c16[:, i, :],
                                        op=mybir.AluOpType.min)
                cur = nxt
            nc.vector.tensor_reduce(mins16[:, t:t + 1], cur,
                                    axis=mybir.AxisListType.X,
                                    op=mybir.AluOpType.min)

        def whole_pass():
            for t in range(TILES):
                tile_body(t)

        if reps == 1:
            whole_pass()
        else:
            with tc.For_i(0, reps, 1):
                whole_pass()

        m16f = mpool.tile([PT, TILES], f32)
        nc.scalar.copy(out=m16f, in_=mins16)
        both = mpool.tile([PT, TILES], f32)
        nc.vector.tensor_tensor(out=both, in0=mins32, in1=m16f,
                                op=mybir.AluOpType.min)
        nc.sync.dma_start(out=out[:, :], in_=both)

    nc.compile()
    _NC_CACHE[key] = nc
    return nc


def _split16(x):
    hi = x.astype(np.float16)
    lo = (x - hi.astype(np.float32)).astype(np.float16)
    return hi, lo


def _make_in_maps(vertices, pc):
    """vertices [B,3,128,64] f32, pc [B,3,M] f32 -> list of 8 in_maps."""
    in_maps = []
    onesq = np.ones((1, MQ), np.float16)
    onesn = np.ones((1, N), np.float16)
    for b in range(B):
        v = vertices[b].reshape(3, N).astype(np.float32)
        m2v = -2.0 * v
        m2v_hi, m2v_lo = _split16(m2v)
        V2 = (v.astype(np.float64) ** 2).sum(0).astype(np.float32)
        V2_hi, V2_lo = _split16(V2)
        rhs_b = np.concatenate(
            [m2v_hi, m2v_lo, m2v_hi, V2_hi[None], V2_lo[None], onesn, onesn],
            axis=0).astype(np.float16)
        rhs_b = np.ascontiguousarray(rhs_b)
        for h in range(2):
            p = pc[b, :, h * MQ:(h + 1) * MQ].astype(np.float32)
            p_hi, p_lo = _split16(p)
            P2 = (p.astype(np.float64) ** 2).sum(0).astype(np.float32)
            P2_hi, P2_lo = _split16(P2)
            lhsT_c = np.concatenate(
                [p_hi, p_hi, p_lo, onesq, onesq, P2_hi[None], P2_lo[None]],
                axis=0).astype(np.float16)
            in_maps.append({"lhsT": np.ascontiguousarray(lhsT_c),
                            "rhs": rhs_b})
    return in_maps


def _get_runner():
    """Build the kernel once and return a cached callable that executes it
    on all 8 cores via a persistently-jitted shard_map (adapted from
    concourse.bass2jax.run_bass_via_pjrt, which re-jits on every call)."""
    if "runner" in _NC_CACHE:
        return _NC_CACHE["runner"]

    import jax
    from jax.experimental.shard_map import shard_map
    from jax.sharding import Mesh, PartitionSpec
    import concourse.mybir as _mybir
    from concourse import bass2jax

    nc = _build()
    bass2jax.install_neuronx_cc_hook()

    partition_name = nc.partition_id_tensor.name if nc.partition_id_tensor else None
    in_names, out_names, out_avals, zero_shapes = [], [], [], []
    for alloc in nc.m.functions[0].allocations:
        if not isinstance(alloc, _mybir.MemoryLocationSet):
            continue
        name = alloc.memorylocations[0].name
        if alloc.kind == "ExternalInput":
            if name != partition_name:
                in_names.append(name)
        elif alloc.kind == "ExternalOutput":
            shape = tuple(alloc.tensor_shape)
            dtype = _mybir.dt.np(alloc.dtype)
            out_names.append(name)
            out_avals.append(jax.core.ShapedArray(shape, dtype))
            zero_shapes.append((shape, dtype))
    n_params = len(in_names)
    n_outs = len(out_names)
    all_in_names = tuple(in_names + out_names + ([partition_name] if partition_name else []))

    def _body(*args):
        operands = list(args)
        if partition_name is not None:
            operands.append(bass2jax.partition_id_tensor())
        outs = bass2jax._bass_exec_p.bind(
            *operands,
            out_avals=tuple(out_avals),
            in_names=all_in_names,
            out_names=tuple(out_names),
            lowering_input_output_aliases=(),
            sim_require_finite=True,
            sim_require_nnan=True,
            nc=nc,
        )
        return tuple(outs)

    devices = jax.devices()[:NCORES]
    mesh = Mesh(np.asarray(devices), ("core",))
    donate = tuple(range(n_params, n_params + n_outs))
    sharded = jax.jit(
        shard_map(_body, mesh=mesh,
                  in_specs=(PartitionSpec("core"),) * (n_params + n_outs),
                  out_specs=(PartitionSpec("core"),) * n_outs,
                  check_rep=False),
        donate_argnums=donate, keep_unused=True)

    def run(in_maps):
        concat_in = [
            np.concatenate([np.asarray(m[name]) for m in in_maps], axis=0)
            for name in in_names
        ]
        concat_zeros = [
            np.zeros((NCORES * s[0], *s[1:]), d) for (s, d) in zero_shapes
        ]
        out_arrs = jax.block_until_ready(sharded(*concat_in, *concat_zeros))
        return [
            {name: np.asarray(out_arrs[i]).reshape(NCORES, *out_avals[i].shape)[c]
             for i, name in enumerate(out_names)}
            for c in range(NCORES)
        ]

    _NC_CACHE["runner"] = run
    return run


def _run_device(in_maps):
    return _get_runner()(in_maps)


def kernel(vertices, pc):
    vertices = np.asarray(vertices, dtype=np.float32)
    pc = np.asarray(pc, dtype=np.float32)
    in_maps = _make_in_maps(vertices, pc)
    results = _run_device(in_maps)

    dist2 = np.empty((B, M), np.float64)
    for b in range(B):
        for h in range(2):
            core = b * 2 + h
            o = results[core]["out"]              # [128, TILES]
            mins = o.T.reshape(MQ)                # point index = t*128 + m
            dist2[b, h * MQ:(h + 1) * MQ] = mins

    valid = ~np.all(pc == 0.0, axis=1)            # [B, M]
    valid_f = valid.astype(np.float64)
    per_item = (dist2 * valid_f).sum(axis=1) / valid_f.sum(axis=1)
    return np.float32(per_item.mean())
